# revision 1
# baseline (speedup 1.0000x reference)
"""Trainium2 Bass kernel for nn_ActivationQuantizer (quantize + im2col + topk row/col masking).

Pipeline (8 NeuronCores, data-parallel over batch B=8, one image per core):
  Host:     global min/max (2-scalar reduction) -> scale, exact zero boundary X0.
  Launch B: per-core nonzero-count stats (row sums, col-sum tree-fold,
            corners, per-pixel channel-sum map via ones-matmul)
            -> host: all-reduce row counts, 3x3 box-sum col counts,
               sort -> thresholds r1, r2 (the cross-device "all-reduce
               then threshold" step from the sharding hint).
  Launch C: per-core quantize + 9-shift im2col expansion with row/col
            masks folded into one scalar_tensor_tensor per plane slice,
            writes [1152, 3136] f32 at ~HBM write rate.
  Host:     interleave per-core outputs into [1152, 25088] (batch-minor).

Exactness strategy: the row/col masks depend on integer nonzero counts of
q = round(x/scale). round(t)==0 <=> |t| <= 0.5 (RNE), and f32 division is
monotone, so q!=0 <=> |x| > X0 where X0 = largest f32 with fl(X0/scale) <= 0.5
(found on host by exact f32 search). The device tests |x| > X0 with exact
comparisons, so counts match the jax reference bit-exactly. Output q values use
the f32 magic-number RNE trick (x*inv + M) - M; an off-by-one ULP there only
perturbs a handful of element values by ~scale, never the masks.
"""

import sys

if "/opt/trn_rl_repo" not in sys.path:
    sys.path.insert(0, "/opt/trn_rl_repo")

import math

import ml_dtypes
import numpy as np

import concourse.bacc as bacc
import concourse.mybir as mybir
from concourse.tile import TileContext
from concourse.bass_utils import run_bass_kernel_spmd

F32 = mybir.dt.float32
BF16 = mybir.dt.bfloat16
ALU = mybir.AluOpType
AX = mybir.AxisListType

B, C, H, W = 8, 128, 56, 56
HW = H * W              # 3136
PH, PW = H + 2, W + 2   # 58
PHW = PH * PW           # 3364
NO = 9                  # 3x3 filter offsets
R = C * NO              # 1152 output rows
L = B * HW              # 25088 output cols
RATIO = (0.2, 0.2)
MAGIC = float(np.float32(12582912.0))  # 1.5 * 2**23: f32 RNE rounding constant

CORES = list(range(8))

_NC_CACHE = {}

LAST_PROFILE = {}


def _nc_counts():
    nc = bacc.Bacc()
    x = nc.dram_tensor("x", [C, HW], F32, kind="ExternalInput")
    thr = nc.dram_tensor("thr", [C, 1], F32, kind="ExternalInput")
    # stats layout per channel: RS[0:56] | CS[56:112] | q00,q05,q50,q55 [112:116] | T [116]
    stats = nc.dram_tensor("stats", [C, 117], F32, kind="ExternalOutput")
    smap = nc.dram_tensor("smap", [1, HW], F32, kind="ExternalOutput")
    with TileContext(nc) as tc:
        with (
            tc.tile_pool(name="p", bufs=1) as pool,
            tc.tile_pool(name="ps", bufs=4, space="PSUM") as psp,
        ):
            xt = pool.tile([C, HW], F32)
            th = pool.tile([C, 1], F32)
            nc.sync.dma_start(out=th[:, :], in_=thr[:, :])
            absx = pool.tile([C, HW], F32)
            nzb = pool.tile([C, HW], BF16)
            st = pool.tile([C, 117], F32)
            nz3 = nzb[:, :].rearrange("c (h w) -> c h w", h=H)
            NCH = 4
            RCH = H // NCH  # 14 rows per chunk
            CH = RCH * W
            engs = [nc.sync, nc.scalar, nc.sync, nc.scalar]
            for j in range(NCH):
                sl = slice(j * CH, (j + 1) * CH)
                engs[j].dma_start(out=xt[:, sl], in_=x[:, sl])
                # nz = (|x| > X0) as bf16 0/1 (exact):
                # |x| via sign-bit clear on the int32 view, then compare
                nc.vector.tensor_scalar(
                    absx[:, sl].bitcast(mybir.dt.uint32),
                    xt[:, sl].bitcast(mybir.dt.uint32),
                    0x7FFFFFFF,
                    None,
                    ALU.bitwise_and,
                )
                nc.vector.tensor_scalar(
                    nzb[:, sl], absx[:, sl], th[:, 0:1], None, ALU.is_gt
                )
                nc.vector.tensor_reduce(
                    st[:, j * RCH : (j + 1) * RCH],
                    nz3[:, j * RCH : (j + 1) * RCH, :],
                    axis=AX.X,
                    op=ALU.add,
                )
            # CS[c,w] = sum_h nz[c,h,w]: contiguous tree-fold over h rows
            # (56 = 8*7): fold 28+28, 14+14, 7+7 -> [7,56], then reduce the
            # 7 rows via a strided-X reduce over a small [c,56,7] view.
            fold = pool.tile([C, 28 * W], BF16)
            nc.vector.tensor_tensor(
                fold[:, : 28 * W], nzb[:, : 28 * W], nzb[:, 28 * W :], ALU.add
            )
            nc.vector.tensor_tensor(
                fold[:, : 14 * W], fold[:, : 14 * W], fold[:, 14 * W : 28 * W], ALU.add
            )
            nc.vector.tensor_tensor(
                fold[:, : 7 * W], fold[:, : 7 * W], fold[:, 7 * W : 14 * W], ALU.add
            )
            f7 = fold[:, : 7 * W].rearrange("c (h w) -> c w h", h=7)
            nc.vector.tensor_reduce(st[:, 56:112], f7, axis=AX.X, op=ALU.add)
            nc.vector.tensor_copy(st[:, 112:114], nzb[:, 0 : W : W - 1])
            nc.vector.tensor_copy(st[:, 114:116], nzb[:, (H - 1) * W : HW : W - 1])
            nc.vector.tensor_reduce(st[:, 116:117], st[:, 0:56], axis=AX.X, op=ALU.add)
            # channel-sum map S[hw] = sum_c nz[c, hw] via ones-matmul (PSUM 512/bank)
            ones = pool.tile([C, 1], BF16)
            nc.vector.memset(ones[:, :], 1.0)
            ssb = pool.tile([1, HW], F32)
            nchunk = (HW + 511) // 512
            for j in range(nchunk):
                n = min(512, HW - j * 512)
                pt = psp.tile([1, 512], F32, tag="pt")
                nc.tensor.matmul(
                    pt[0:1, 0:n],
                    ones[:, 0:1],
                    nzb[:, j * 512 : j * 512 + n],
                    start=True,
                    stop=True,
                )
                nc.scalar.copy(ssb[0:1, j * 512 : j * 512 + n], pt[0:1, 0:n])
            nc.sync.dma_start(out=stats[:, :], in_=st[:, :])
            nc.sync.dma_start(out=smap[:, :], in_=ssb[0:1, :])
    nc.compile()
    return nc


def _nc_expand():
    nc = bacc.Bacc()
    x = nc.dram_tensor("x", [C, HW], F32, kind="ExternalInput")
    inv = nc.dram_tensor("inv", [C, 1], F32, kind="ExternalInput")
    rs9 = nc.dram_tensor("rs9", [C, NO], F32, kind="ExternalInput")
    cm = nc.dram_tensor("cm", [1, HW], BF16, kind="ExternalInput")
    out = nc.dram_tensor("out", [R, HW], F32, kind="ExternalOutput")
    outv = out[:, :].rearrange("(c o) l -> c o l", o=NO)
    with TileContext(nc) as tc:
        with (
            tc.tile_pool(name="p", bufs=1) as pool,
            tc.tile_pool(name="pp", bufs=5) as pp,
            tc.tile_pool(name="ps", bufs=4, space="PSUM") as psp,
        ):
            xt = pool.tile([C, HW], F32)
            invt = pool.tile([C, 1], F32)
            nc.sync.dma_start(out=invt[:, :], in_=inv[:, :])
            rst = pool.tile([C, NO], F32)
            nc.sync.dma_start(out=rst[:, :], in_=rs9[:, :])
            cmsrc = pool.tile([1, HW], BF16)
            nc.sync.dma_start(out=cmsrc[:, :], in_=cm[:, :])
            # uneven split: chunk 1 = rows [0,30) alone on the two HWDGE
            # queues (queued followers would interleave and delay its
            # completion); chunk 2 rides the independent SWDGE path.
            RSPLIT = 30
            nc.sync.dma_start(out=xt[:, : 15 * W], in_=x[:, : 15 * W])
            nc.scalar.dma_start(
                out=xt[:, 15 * W : RSPLIT * W], in_=x[:, 15 * W : RSPLIT * W]
            )
            nc.gpsimd.dma_start(out=xt[:, RSPLIT * W :], in_=x[:, RSPLIT * W :])
            onesb = pool.tile([1, C], BF16)
            nc.vector.memset(onesb[:, :], 1.0)
            cmt = pool.tile([C, HW], F32)
            nchunk = (HW + 511) // 512
            for j in range(nchunk):
                n = min(512, HW - j * 512)
                pcm = psp.tile([C, 512], F32, tag="pcm")
                nc.tensor.matmul(
                    pcm[:, 0:n],
                    onesb[0:1, :],
                    cmsrc[0:1, j * 512 : j * 512 + n],
                    start=True,
                    stop=True,
                )
                nc.scalar.copy(cmt[:, j * 512 : j * 512 + n], pcm[:, 0:n])
            cm3 = cmt[:, :].rearrange("c (h w) -> c h w", h=H)
            # padded quantized image qp[c, 58, 58]; zero only the border ring
            qp = pool.tile([C, PHW], F32)
            qv = qp[:, :].rearrange("c (a b) -> c a b", a=PH)
            nc.vector.memset(qv[:, 0, :], 0.0)
            nc.vector.memset(qv[:, PH - 1, :], 0.0)
            nc.vector.memset(qv[:, 1 : PH - 1, 0], 0.0)
            nc.vector.memset(qv[:, 1 : PH - 1, PW - 1], 0.0)
            qpi = qv[:, 1 : 1 + H, 1 : 1 + W]
            # q = RNE(x * inv) via magic add/sub; linear intermediate, and
            # computed in two row-chunks matching the x load split
            ql = pool.tile([C, HW], F32)
            ql3 = ql[:, :].rearrange("c (h w) -> c h w", h=H)

            def q_rows(r0, r1):
                sl = slice(r0 * W, r1 * W)
                nc.vector.tensor_scalar(
                    ql[:, sl], xt[:, sl], invt[:, 0:1], MAGIC, ALU.mult, ALU.add
                )
                nc.vector.tensor_scalar(
                    qpi[:, r0:r1, :], ql3[:, r0:r1, :], 1.0, -MAGIC,
                    ALU.mult, ALU.add,
                )

            HALF = H // 2
            ne = 0

            def plane_part(o, r0, nr):
                nonlocal ne
                fi, fj = divmod(o, 3)
                pl = pp.tile([C, nr * W], F32, tag="pl", name=f"pl{o}_{r0}")
                pl3 = pl[:, :].rearrange("c (h w) -> c h w", h=nr)
                qs = qv[:, fi + r0 : fi + r0 + nr, fj : fj + W]
                nc.vector.scalar_tensor_tensor(
                    pl3,
                    qs,
                    rst[:, o : o + 1],
                    cm3[:, r0 : r0 + nr, :],
                    ALU.mult,
                    ALU.mult,
                )
                eng = nc.sync if ne % 2 == 0 else nc.scalar
                ne += 1
                eng.dma_start(
                    out=outv[:, o, r0 * W : (r0 + nr) * W], in_=pl[:, :]
                )

            q_rows(0, RSPLIT)          # qp rows [1,31) ready
            for o in range(4):         # top halves need qp rows [0,30)
                plane_part(o, 0, HALF)
            q_rows(RSPLIT, H)          # qp rows [31,57) ready
            for o in range(4):
                plane_part(o, HALF, H - HALF)
            for o in range(4, NO):
                plane_part(o, 0, H)
    nc.compile()
    return nc


def _get(name, builder):
    if name not in _NC_CACHE:
        _NC_CACHE[name] = builder()
    return _NC_CACHE[name]


def _run(nc, in_maps, **kw):
    """run_bass_kernel_spmd with one retry (transient device-wedge insurance)."""
    try:
        return run_bass_kernel_spmd(nc, in_maps, core_ids=CORES, **kw)
    except Exception:
        import time

        time.sleep(2.0)
        return run_bass_kernel_spmd(nc, in_maps, core_ids=CORES, **kw)


def _find_x0(scale):
    """Largest f32 v with fl(v/scale) <= 0.5 (q==0 boundary under RNE)."""
    s = np.float32(scale)
    half = np.float32(0.5)
    v = np.float32(half * s)
    inf32 = np.float32(np.inf)
    while np.float32(v) / s > half:
        v = np.nextafter(v, -inf32, dtype=np.float32)
    while True:
        nv = np.nextafter(v, inf32, dtype=np.float32)
        if np.float32(nv) / s <= half:
            v = nv
        else:
            break
    return np.float32(v)


def kernel(x, bits, _trace=False):
    bits = int(bits)
    x = np.ascontiguousarray(np.asarray(x, dtype=np.float32))
    assert x.shape == (B, C, H, W), x.shape
    xb = x.reshape(B, C, HW)

    trace_kw = {"trace": True} if _trace else {}
    LAST_PROFILE.clear()

    # ---- global min/max (2-scalar reduction, host) -> scale, X0 ----
    mn = np.float32(np.min(x))
    mx = np.float32(np.max(x))
    scale = np.float32((mx - mn) / np.float32(2**bits - 1))
    inv_scale = np.float32(np.float32(1.0) / scale)
    x0 = _find_x0(scale)

    # ---- Launch B: nonzero-count stats ----
    ncB = _get("counts", _nc_counts)
    thr = np.full((C, 1), x0, dtype=np.float32)
    resB = _run(ncB, [{"x": xb[b], "thr": thr} for b in range(B)], **trace_kw)
    if _trace:
        LAST_PROFILE["B_ns"] = resB.exec_time_ns

    # host: per-core row counts nzr_b[c, fi, fj] and col counts nzc_b[oi, oj]
    nzr = np.zeros((C, 3, 3), dtype=np.int64)
    nzc_per_core = []
    for b in range(B):
        st = resB.results[b]["stats"].astype(np.float64)
        RS = st[:, 0:56]
        CS = st[:, 56:112]
        q00, q05 = st[:, 112], st[:, 113]
        q50, q55 = st[:, 114], st[:, 115]
        T = st[:, 116]
        row_excl = [RS[:, 55], np.zeros(C), RS[:, 0]]   # fi = 0,1,2
        col_excl = [CS[:, 55], np.zeros(C), CS[:, 0]]   # fj = 0,1,2
        corner = {
            (0, 0): q55, (0, 2): q50,
            (2, 0): q05, (2, 2): q00,
        }
        for fi in range(3):
            for fj in range(3):
                v = T - row_excl[fi] - col_excl[fj] + corner.get((fi, fj), 0.0)
                nzr[:, fi, fj] += np.rint(v).astype(np.int64)
        S = resB.results[b]["smap"].reshape(H, W).astype(np.float64)
        Sp = np.pad(S, 1)
        nzc = np.zeros((H, W), dtype=np.float64)
        for di in range(3):
            for dj in range(3):
                nzc += Sp[di : di + H, dj : dj + W]
        nzc_per_core.append(np.rint(nzc).astype(np.int64).reshape(HW))

    nzr_flat = nzr.reshape(R)  # r = c*9 + fi*3 + fj
    r1 = np.sort(nzr_flat)[int(math.ceil(R * RATIO[0]))]
    nzc_all = np.concatenate(nzc_per_core)
    r2 = np.sort(nzc_all)[int(math.ceil(L * RATIO[1]))]

    rowscale = np.where(nzr_flat >= r1, scale, np.float32(0.0)).astype(np.float32)
    rs9 = np.ascontiguousarray(rowscale.reshape(C, NO))
    invrep = np.full((C, 1), inv_scale, dtype=np.float32)

    # ---- Launch C: masked im2col expansion ----
    ncC = _get("expand", _nc_expand)
    in_maps = []
    for b in range(B):
        cm_b = (
            (nzc_per_core[b] >= r2)
            .astype(ml_dtypes.bfloat16)
            .reshape(1, HW)
        )
        in_maps.append({"x": xb[b], "inv": invrep, "rs9": rs9, "cm": cm_b})
    resC = _run(ncC, in_maps, **trace_kw)
    if _trace:
        LAST_PROFILE["C_ns"] = resC.exec_time_ns

    outs = [resC.results[b]["out"] for b in range(B)]  # each [R, HW]
    full = np.stack(outs, axis=2).reshape(R, L)
    return full



# revision 8
# speedup vs baseline: 1.0148x; 1.0148x over previous
"""Trainium2 Bass kernel for nn_ActivationQuantizer (quantize + im2col + topk row/col masking).

Pipeline (8 NeuronCores, data-parallel over batch B=8, one image per core):
  Host:     global min/max -> scale, exact zero boundary X0.
  Launch B: per-core nonzero-count stats (row sums via chunked reduces,
            col sums via tree-fold, corners, per-pixel channel-sum map via
            ones-matmul into one [1,HW] PSUM tile) + quantize q=RNE(x/scale)
            on the ACT engine, written to DRAM as fp16 (exact small ints).
  Host:     all-reduce row counts (inclusion-exclusion over 9 offsets),
            3x3 box-sum col counts, global sort -> thresholds r1, r2
            (the cross-device "all-reduce then threshold" step).
  Launch C: per-core masked im2col expansion, all-flat DVE ops:
            colmask broadcast via K=1 matmul into PSUM (read directly by
            TT), per plane TS (q_shifted * rowmask01, 2x fp16) + TT
            (* colmask, reading PSUM f32), 9 x [C,HW] fp16 writes.
  Host:     zero structural pad borders, interleave cores (l = hw*B + b),
            single f32 multiply by scale (reproduces the reference's one
            f32 rounding -> output matches reference bit-exactly up to
            ~1-per-400k FMA-boundary quantization values).

Exactness: masks depend on integer nonzero counts of q = round(x/scale).
round(t)==0 <=> |t| <= 0.5 (RNE) and f32 division is monotone, so
q!=0 <=> |x| > X0 where X0 = largest f32 with fl(X0/scale) <= 0.5 (exact
host-side f32 search). The device tests |x| > X0 with exact compares, so
counts and masks match the jax reference bit-exactly. Output values use
q from the ACT engine's fused (x*inv + MAGIC) - MAGIC (FMA rounding can
differ from the reference's division by 1 ulp at half-integer boundaries:
~1 element in 4e5, value error +-scale, negligible vs the 2e-2 gate).
"""

import sys

if "/opt/trn_rl_repo" not in sys.path:
    sys.path.insert(0, "/opt/trn_rl_repo")

import math

import numpy as np

import concourse.bacc as bacc
import concourse.mybir as mybir
from concourse.tile import TileContext
from concourse.bass_utils import run_bass_kernel_spmd

F32 = mybir.dt.float32
F16 = mybir.dt.float16
BF16 = mybir.dt.bfloat16
U32 = mybir.dt.uint32
ALU = mybir.AluOpType
AX = mybir.AxisListType
ACTF = mybir.ActivationFunctionType

B, C, H, W = 8, 128, 56, 56
HW = H * W              # 3136
NO = 9                  # 3x3 filter offsets
R = C * NO              # 1152 output rows
L = B * HW              # 25088 output cols
RATIO = (0.2, 0.2)
MAGIC = float(np.float32(12582912.0))  # 1.5 * 2**23: f32 RNE rounding constant
MARG = 64               # qt margin elements on each side (covers offsets +-57)

CORES = list(range(8))

_NC_CACHE = {}

LAST_PROFILE = {}


def _nc_stats():
    """Launch B: nz stats + fp16 quantized image."""
    nc = bacc.Bacc()
    x = nc.dram_tensor("x", [C, HW], F32, kind="ExternalInput")
    thr = nc.dram_tensor("thr", [C, 1], F32, kind="ExternalInput")
    inv = nc.dram_tensor("inv", [C, 1], F32, kind="ExternalInput")
    # stats per channel: RS[0:56] | CS[56:112] | q00,q05,q50,q55 [112:116] | T [116]
    stats = nc.dram_tensor("stats", [C, 117], F32, kind="ExternalOutput")
    smap = nc.dram_tensor("smap", [1, HW], F32, kind="ExternalOutput")
    q = nc.dram_tensor("q", [C, HW], F16, kind="ExternalOutput")
    with TileContext(nc) as tc:
        with (
            tc.tile_pool(name="p", bufs=1) as pool,
            tc.tile_pool(name="ps", bufs=1, space="PSUM") as psp,
        ):
            xt = pool.tile([C, HW], F32)
            th = pool.tile([C, 1], F32)
            invt = pool.tile([C, 1], F32)
            nc.sync.dma_start(out=th[:, :], in_=thr[:, :])
            nc.sync.dma_start(out=invt[:, :], in_=inv[:, :])
            absx = pool.tile([C, HW], F32)
            nzb = pool.tile([C, HW], BF16)
            qlt = pool.tile([C, HW], F32)
            qt16 = pool.tile([C, HW], F16)
            st = pool.tile([C, 117], F32)
            ones = pool.tile([C, 1], BF16)
            nc.vector.memset(ones[:, :], 1.0)
            nz3 = nzb[:, :].rearrange("c (h w) -> c h w", h=H)
            pt = psp.tile([1, HW], F32)

            NCH = 4
            RCH = H // NCH  # 14 rows per chunk
            CH = RCH * W    # 784 elements per chunk
            # matmul j covers nzb cols [512j, 512j+n): emit after the nz
            # chunk that completes its range (chunk k covers [784k, 784k+784))
            mm_after = {0: [0], 1: [1, 2], 2: [3], 3: [4, 5, 6]}
            ldeng = [nc.sync, nc.scalar, nc.sync, nc.scalar]
            for k in range(NCH):  # issue all loads up front
                sl = slice(k * CH, (k + 1) * CH)
                ldeng[k].dma_start(out=xt[:, sl], in_=x[:, sl])
            for k in range(NCH):
                sl = slice(k * CH, (k + 1) * CH)
                # nz = (|x| > X0) exact: sign-bit clear then compare
                nc.vector.tensor_scalar(
                    absx[:, sl].bitcast(U32), xt[:, sl].bitcast(U32),
                    0x7FFFFFFF, None, ALU.bitwise_and,
                )
                nc.vector.tensor_scalar(
                    nzb[:, sl], absx[:, sl], th[:, 0:1], None, ALU.is_gt
                )
                nc.vector.tensor_reduce(
                    st[:, k * RCH:(k + 1) * RCH],
                    nz3[:, k * RCH:(k + 1) * RCH, :],
                    axis=AX.X, op=ALU.add,
                )
                # ACT engine: q = RNE(x*inv) via magic add/sub, fp16 out
                nc.scalar.activation(qlt[:, sl], xt[:, sl], ACTF.Copy,
                                     bias=MAGIC, scale=invt[:, 0:1])
                nc.scalar.activation(qt16[:, sl], qlt[:, sl], ACTF.Copy,
                                     bias=-MAGIC, scale=1.0)
                nc.gpsimd.dma_start(out=q[:, sl], in_=qt16[:, sl])
                # smap partial matmuls: S[hw] = sum_c nz[c, hw]
                for j in mm_after[k]:
                    n = min(512, HW - j * 512)
                    nc.tensor.matmul(
                        pt[0:1, j * 512:j * 512 + n], ones[:, 0:1],
                        nzb[:, j * 512:j * 512 + n], start=True, stop=True,
                    )
            # CS[c,w] = sum_h nz[c,h,w]: tree-fold over h (56 = 8*7)
            fold = pool.tile([C, 28 * W], BF16)
            nc.vector.tensor_tensor(
                fold[:, :28 * W], nzb[:, :28 * W], nzb[:, 28 * W:], ALU.add
            )
            nc.vector.tensor_tensor(
                fold[:, :14 * W], fold[:, :14 * W], fold[:, 14 * W:28 * W], ALU.add
            )
            nc.vector.tensor_tensor(
                fold[:, :7 * W], fold[:, :7 * W], fold[:, 7 * W:14 * W], ALU.add
            )
            f7 = fold[:, :7 * W].rearrange("c (h w) -> c w h", h=7)
            nc.vector.tensor_reduce(st[:, 56:112], f7, axis=AX.X, op=ALU.add)
            nc.vector.tensor_copy(st[:, 112:114], nzb[:, 0:W:W - 1])
            nc.vector.tensor_copy(st[:, 114:116], nzb[:, (H - 1) * W:HW:W - 1])
            nc.vector.tensor_reduce(st[:, 116:117], st[:, 0:56], axis=AX.X, op=ALU.add)
            ssb = pool.tile([1, HW], F32)
            nc.scalar.copy(ssb[:, :], pt[:, :])
            nc.sync.dma_start(out=stats[:, :], in_=st[:, :])
            nc.scalar.dma_start(out=smap[:, :], in_=ssb[:, :])
    nc.compile()
    return nc


def _nc_expand():
    """Launch C: masked im2col expansion, all-flat fp16 DVE ops."""
    nc = bacc.Bacc()
    q = nc.dram_tensor("q", [C, HW], F16, kind="ExternalInput")
    cm = nc.dram_tensor("cm", [1, HW], F16, kind="ExternalInput")
    rm = nc.dram_tensor("rm", [C, NO], F32, kind="ExternalInput")
    out = nc.dram_tensor("out", [R, HW], F16, kind="ExternalOutput")
    outv = out[:, :].rearrange("(c o) l -> c o l", o=NO)
    QT = HW + 2 * MARG
    with TileContext(nc) as tc:
        with (
            tc.tile_pool(name="p", bufs=1) as pool,
            tc.tile_pool(name="pt", bufs=9) as ptp,
            tc.tile_pool(name="pp", bufs=4) as ppp,
            tc.tile_pool(name="ps", bufs=1, space="PSUM") as psp,
        ):
            qt = pool.tile([C, QT], F16)
            cmsrc = pool.tile([1, HW], F16)
            rmt = pool.tile([C, NO], F32)
            onesb = pool.tile([1, C], F16)
            nc.vector.memset(qt[:, 0:MARG], 0.0)
            nc.vector.memset(qt[:, MARG + HW:], 0.0)
            nc.vector.memset(onesb[:, :], 1.0)
            HALF = (HW // 2) // W * W  # 1568
            nc.scalar.dma_start(out=cmsrc[:, :], in_=cm[:, :])
            nc.scalar.dma_start(out=rmt[:, :], in_=rm[:, :])
            nc.sync.dma_start(out=qt[:, MARG:MARG + HALF], in_=q[:, 0:HALF])
            nc.scalar.dma_start(out=qt[:, MARG + HALF:MARG + HW], in_=q[:, HALF:])
            # colmask broadcast to all partitions: K=1 matmul into PSUM
            # (bank-aligned 512 chunks); plane TTs read it as f32 directly.
            pcm = psp.tile([C, HW], F32)
            for j in range(7):
                n = min(512, HW - j * 512)
                nc.tensor.matmul(
                    pcm[:, j * 512:j * 512 + n], onesb[0:1, :],
                    cmsrc[0:1, j * 512:j * 512 + n], start=True, stop=True,
                )
            # planes: off = 56*(fi-1) + (fj-1); out[k] = q[k+off]*rm*cm[k]
            # TS on vector for planes 0..3, on ACT for 4..8; TT always
            # vector. Write queues avoid ACT (its stream is busy with TS
            # until late) except the final plane 7 issued after TS8.
            wr = [nc.sync, nc.gpsimd, nc.sync, nc.gpsimd, nc.sync,
                  nc.gpsimd, nc.gpsimd, nc.scalar, nc.gpsimd]
            tmp = {}

            def ts(o, eng):
                fi, fj = divmod(o, 3)
                off = (fi - 1) * W + (fj - 1)
                t = ptp.tile([C, HW], F16, tag="tmp", name=f"tmp{o}")
                src = qt[:, MARG + off:MARG + off + HW]
                if eng == "v":
                    nc.vector.tensor_scalar(t[:, :], src, rmt[:, o:o + 1],
                                            None, ALU.mult)
                else:
                    nc.scalar.activation(t[:, :], src, ACTF.Copy, bias=0.0,
                                         scale=rmt[:, o:o + 1])
                tmp[o] = t

            def tt(o):
                pl = ppp.tile([C, HW], F16, tag="pl", name=f"pl{o}")
                nc.vector.tensor_tensor(pl[:, :], tmp[o][:, :], pcm[:, :], ALU.mult)
                wr[o].dma_start(out=outv[:, o, :], in_=pl[:, :])

            # ACT-side TS for late planes (emitted first so ACT runs ahead)
            for o in (4, 5, 6, 7, 8):
                ts(o, "a")
            ts(0, "v")
            ts(1, "v")
            tt(0)
            ts(2, "v")
            tt(1)
            ts(3, "v")
            for o in range(2, NO):
                tt(o)
    nc.compile()
    return nc


def _get(name, builder):
    if name not in _NC_CACHE:
        _NC_CACHE[name] = builder()
    return _NC_CACHE[name]


def _run(nc, in_maps, **kw):
    """run_bass_kernel_spmd with one retry (transient device-wedge insurance)."""
    try:
        return run_bass_kernel_spmd(nc, in_maps, core_ids=CORES, **kw)
    except Exception:
        import time

        time.sleep(2.0)
        return run_bass_kernel_spmd(nc, in_maps, core_ids=CORES, **kw)


def _find_x0(scale):
    """Largest f32 v with fl(v/scale) <= 0.5 (q==0 boundary under RNE)."""
    s = np.float32(scale)
    half = np.float32(0.5)
    v = np.float32(half * s)
    inf32 = np.float32(np.inf)
    while np.float32(v) / s > half:
        v = np.nextafter(v, -inf32, dtype=np.float32)
    while True:
        nv = np.nextafter(v, inf32, dtype=np.float32)
        if np.float32(nv) / s <= half:
            v = nv
        else:
            break
    return np.float32(v)


def kernel(x, bits, _trace=False):
    bits = int(bits)
    x = np.ascontiguousarray(np.asarray(x, dtype=np.float32))
    assert x.shape == (B, C, H, W), x.shape
    xb = x.reshape(B, C, HW)

    trace_kw = {"trace": True} if _trace else {}
    LAST_PROFILE.clear()

    # ---- global min/max (2-scalar reduction, host) -> scale, X0 ----
    mn = np.float32(np.min(x))
    mx = np.float32(np.max(x))
    scale = np.float32((mx - mn) / np.float32(2**bits - 1))
    inv_scale = np.float32(np.float32(1.0) / scale)
    x0 = _find_x0(scale)

    # ---- Launch B: nonzero-count stats + q fp16 ----
    ncB = _get("stats", _nc_stats)
    thr = np.full((C, 1), x0, dtype=np.float32)
    invr = np.full((C, 1), inv_scale, dtype=np.float32)
    resB = _run(ncB, [{"x": xb[b], "thr": thr, "inv": invr} for b in range(B)],
                **trace_kw)
    if _trace:
        LAST_PROFILE["B_ns"] = resB.exec_time_ns

    # host: per-core row counts nzr[c, fi, fj] and col counts nzc[hw]
    nzr = np.zeros((C, 3, 3), dtype=np.int64)
    nzc_per_core = []
    qs = []
    for b in range(B):
        st = resB.results[b]["stats"].astype(np.float64)
        RS = st[:, 0:56]
        CS = st[:, 56:112]
        q00, q05 = st[:, 112], st[:, 113]
        q50, q55 = st[:, 114], st[:, 115]
        T = st[:, 116]
        row_excl = [RS[:, 55], np.zeros(C), RS[:, 0]]   # fi = 0,1,2
        col_excl = [CS[:, 55], np.zeros(C), CS[:, 0]]   # fj = 0,1,2
        corner = {
            (0, 0): q55, (0, 2): q50,
            (2, 0): q05, (2, 2): q00,
        }
        for fi in range(3):
            for fj in range(3):
                v = T - row_excl[fi] - col_excl[fj] + corner.get((fi, fj), 0.0)
                nzr[:, fi, fj] += np.rint(v).astype(np.int64)
        S = resB.results[b]["smap"].reshape(H, W).astype(np.float64)
        Sp = np.pad(S, 1)
        nzc = np.zeros((H, W), dtype=np.float64)
        for di in range(3):
            for dj in range(3):
                nzc += Sp[di:di + H, dj:dj + W]
        nzc_per_core.append(np.rint(nzc).astype(np.int64).reshape(HW))
        qs.append(resB.results[b]["q"])

    nzr_flat = nzr.reshape(R)  # r = c*9 + fi*3 + fj
    r1 = np.sort(nzr_flat)[int(math.ceil(R * RATIO[0]))]
    nzc_all = np.concatenate(nzc_per_core)
    r2 = np.sort(nzc_all)[int(math.ceil(L * RATIO[1]))]

    rm9 = np.ascontiguousarray(
        (nzr_flat >= r1).astype(np.float32).reshape(C, NO))

    # ---- Launch C: masked im2col expansion (unscaled fp16 integers) ----
    ncC = _get("expand", _nc_expand)
    in_maps = []
    for b in range(B):
        cm_b = (nzc_per_core[b] >= r2).astype(np.float16).reshape(1, HW)
        in_maps.append({"q": qs[b], "cm": cm_b, "rm": rm9})
    resC = _run(ncC, in_maps, **trace_kw)
    if _trace:
        LAST_PROFILE["C_ns"] = resC.exec_time_ns

    # ---- host: structural pad borders, interleave, single f32 scale ----
    outs = np.empty((R, HW, B), dtype=np.float16)
    for b in range(B):
        outs[:, :, b] = resC.results[b]["out"]
    ov = outs.reshape(C, NO, H, W, B)
    ov[:, 0:3, 0, :, :] = 0        # fi = 0 -> top row is pad
    ov[:, 6:9, H - 1, :, :] = 0    # fi = 2 -> bottom row is pad
    ov[:, 0::3, :, 0, :] = 0       # fj = 0 -> left col is pad
    ov[:, 2::3, :, W - 1, :] = 0   # fj = 2 -> right col is pad
    full = outs.reshape(R, L).astype(np.float32) * scale
    return full
